# revision 20
# baseline (speedup 1.0000x reference)
"""BertBlock (mean-only LN, 16-head attention, relu FF) on 8 trn2 NeuronCores.

Sharding: sequence-parallel. Each core owns S/8 = 512 rows end-to-end:
LN1, QKV projections (ALL 16 heads for its rows), attention for its 512
queries against all 4096 keys, out-proj, residual, LN2 and the FF — all
local. The only collective is ONE bf16 AllGather carrying K^T and V
(V stored row-major with an interleaved ones-column per head so the
softmax denominator falls out of the same PE accumulation). Q^T is
computed while the AllGather is in flight. No ReduceScatter.

exp() is split between the Scalar engine (exact) and the Vector engine
(Schraudolph fast-exp via int16 bit arithmetic, ~3% rel err) so neither
engine gates the PE. Matmuls run bf16 (1 cycle/row); residuals f32.
"""
import sys

sys.path.insert(0, '/opt/trn_rl_repo')

import numpy as np
import concourse.bass as bass
from concourse import bacc
import concourse.mybir as mybir
import concourse.tile as tile
from concourse.masks import make_identity

S = 4096          # sequence length
H = 1024          # hidden
I_ = 4096         # ffn inner
NH = 16           # heads
HD = 64           # head dim
INNER = NH * HD   # 1024
NC = 8            # cores
SM = S // NC      # 512 rows per core
HC = H // 128     # 8 hidden chunks
IC = INNER // 128 # 8 inner chunks
VW = NH * (HD + 2)  # 1056: per head: 64 V cols, ones col, pad col
AGR = INNER + SM * VW // 512  # 2080 rows of the merged AllGather buffer
F32 = mybir.dt.float32
F32R = mybir.dt.float32r
BF16 = mybir.dt.bfloat16
I16 = mybir.dt.int16
AF = mybir.ActivationFunctionType
ALU = mybir.AluOpType
AXX = mybir.AxisListType.X

# Schraudolph fast-exp in bf16 bit space: bf16_bits(exp(s*x)) ~=
# floor(x * (s*128/ln2) + B); B tuned for minimax rel err (~3.3%),
# including the DVE's truncating f32->int16 conversion.
EXP_MUL = 23.083120654223414   # 0.125 * 128 / ln(2)
EXP_BIAS = 16250.89

_CACHE = {}


def build_nc():
    nc = bacc.Bacc(None, target_bir_lowering=False, debug=False)
    P = lambda name, shape, dt=F32: nc.declare_dram_parameter(name, shape, dt, isOutput=False)
    x_m = P("x_m", [SM, H])
    wqkvT = P("wqkvT", [H, 3 * INNER], BF16)   # [h, q|k|v inner cols, all heads]
    bqkv = P("bqkv", [24, 128])                # (3*INNER) reshaped
    owT = P("owT", [INNER, H], BF16)           # o_w.T
    ob = P("ob", [1, H])
    anw, anb = P("anw", [1, H]), P("anb", [1, H])
    fnw, fnb = P("fnw", [1, H]), P("fnb", [1, H])
    ff1wT = P("ff1wT", [H, I_], BF16)
    ff1b = P("ff1b", [32, 128])
    ff2wT = P("ff2wT", [I_, H], BF16)
    ffb2 = P("ffb2", [1, H])
    y = nc.declare_dram_parameter("y", [SM, H], F32, isOutput=True)

    with tile.TileContext(nc) as tc:
        cst = tc.alloc_tile_pool(name="cst", bufs=1)
        dram = tc.alloc_tile_pool(name="dram", bufs=1, space="DRAM")
        xmp = tc.alloc_tile_pool(name="xmp", bufs=1)
        ctxp = tc.alloc_tile_pool(name="ctxp", bufs=1)
        ctxT = [ctxp.tile([128, SM], BF16, tag=f"ctxT{i}", name=f"ctxT{i}")
                for i in range(IC)]
        qkvp = tc.alloc_tile_pool(name="qkvp", bufs=1)
        QT = [qkvp.tile([128, SM], BF16, tag=f"QT{i}", name=f"QT{i}") for i in range(NH)]
        setp = tc.alloc_tile_pool(name="setp", bufs=1)
        ps_set = tc.alloc_tile_pool(name="ps_set", bufs=2, space="PSUM")

        # two half-key-range AllGather buffers: rows 0:1024 = K^T cols half,
        # rows 1024:2080 = V flat (two 528-row v_loc tiles)
        ag_in_a = dram.tile([AGR, 256], BF16)
        ag_out_a = dram.tile([NC, AGR, 256], BF16, addr_space="Shared")
        ag_in_b = dram.tile([AGR, 256], BF16)
        ag_out_b = dram.tile([NC, AGR, 256], BF16, addr_space="Shared")

        # ---- constants ----
        ident = cst.tile([128, 128], F32)
        make_identity(nc, ident)
        ones_f = cst.tile([1, 128], F32)
        nc.gpsimd.memset(ones_f, 1.0)
        ones1 = cst.tile([1, 128], F32R)
        nc.vector.tensor_copy(ones1[:], ones_f[:])
        for h in range(NH):
            z = slice(64, 128) if h % 2 == 0 else slice(0, 64)
            nc.gpsimd.memset(QT[h][z, :], 0.0)

        def load_vec(p):
            t = setp.tile([1, H], F32, tag=f"v_{p.name}")
            nc.sync.dma_start(out=t[:], in_=p[:])
            return t

        vecs = {n: load_vec(p) for n, p in
                [("anw", anw), ("anb", anb), ("fnw", fnw), ("fnb", fnb),
                 ("ob", ob), ("ffb2", ffb2)]}

        def bcast(name, pool=None):
            # [1, H] -> [128, H] broadcast across partitions via PE
            v = vecs[name]
            bc = (pool or cst).tile([128, H], F32, tag=f"bc_{name}", name=f"bc_{name}")
            for hf in range(H // 512):
                ps = ps_set.tile([128, 512], F32)
                nc.tensor.matmul(ps[:], ones_f[0:1, :], v[0:1, hf * 512:(hf + 1) * 512],
                                 start=True, stop=True)
                nc.vector.tensor_copy(bc[:, hf * 512:(hf + 1) * 512], ps[:])
            return bc

        anw_bc, anb_bc = bcast("anw", setp), bcast("anb", setp)
        fnw_bc, fnb_bc = bcast("fnw"), bcast("fnb")
        ob_bc, ffb2_bc = bcast("ob"), bcast("ffb2")

        # qkv bias, per-partition layout: [128, 24] (col j = inner block j)
        bqkv_ld = setp.tile([24, 128], F32)
        nc.sync.dma_start(out=bqkv_ld[:], in_=bqkv[:])
        ps = ps_set.tile([128, 512], F32)
        nc.tensor.transpose(ps[:, 0:24], bqkv_ld[:], ident[0:24, 0:24])
        qkvb_pp = cst.tile([128, 24], F32)
        nc.vector.tensor_copy(qkvb_pp[:], ps[:, 0:24])

        ffb1_ld = setp.tile([32, 128], F32)
        nc.sync.dma_start(out=ffb1_ld[:], in_=ff1b[:])
        ps = ps_set.tile([128, 512], F32)
        nc.tensor.transpose(ps[:, 0:32], ffb1_ld[:], ident[0:32, 0:32])
        ffb1_pp = cst.tile([128, 32], F32)
        nc.vector.tensor_copy(ffb1_pp[:], ps[:, 0:32])
        ps_set.release()

        # ---- load x, LN1 ----
        xm_tiles = []
        for i in range(4):
            t = xmp.tile([128, H], F32, tag=f"xm{i}")
            nc.sync.dma_start(out=t[:], in_=x_m[i * 128:(i + 1) * 128, :])
            xm_tiles.append(t)

        with tc.tile_pool(name="ln1", bufs=1) as lnp, \
             tc.tile_pool(name="ln1s", bufs=3) as lnsp, \
             tc.tile_pool(name="wq", bufs=1) as wqp, \
             tc.tile_pool(name="ps_qkv", bufs=3, space="PSUM") as ps_qkv:
            wq_t = []
            for hc in range(HC):
                t = wqp.tile([128, 3 * INNER], BF16, tag=f"wqkv{hc}")
                nc.sync.dma_start(out=t[:], in_=wqkvT[hc * 128:(hc + 1) * 128, :])
                wq_t.append(t)

            xn_tiles = []
            for i in range(4):
                ns = lnsp.tile([128, 1], F32, tag="negsum")
                nc.vector.reduce_sum(out=ns[:], in_=xm_tiles[i][:], axis=AXX,
                                     negate=True)
                nm = lnsp.tile([128, 1], F32, tag="negmean")
                nc.scalar.mul(nm[:], ns[:], 1.0 / H)
                xn = lnp.tile([128, H], F32, tag=f"xn{i}")
                nc.vector.scalar_tensor_tensor(
                    out=xn[:], in0=xm_tiles[i][:], scalar=nm[:], in1=anw_bc[:],
                    op0=ALU.add, op1=ALU.mult)
                nc.vector.tensor_add(xn[:], xn[:], anb_bc[:])
                xn_tiles.append(xn)
            # transpose -> xnT [h, s] bf16 (si-outer: start as soon as each
            # LN1 row-tile is done)
            xnT = [lnp.tile([128, SM], BF16, tag=f"xnT{hc}", name=f"xnT{hc}")
                   for hc in range(HC)]
            for si in range(4):
                for hc in range(HC):
                    ps = ps_qkv.tile([128, 512], F32, tag="tps", bufs=2)
                    nc.tensor.transpose(ps[:, 0:128],
                                        xn_tiles[si][:, hc * 128:(hc + 1) * 128],
                                        ident[:])
                    nc.vector.tensor_copy(xnT[hc][:, si * 128:(si + 1) * 128],
                                          ps[:, 0:128])

            # ---- K^T first (feeds the AllGather ASAP) ----
            for ic in range(IC):
                ps = ps_qkv.tile([128, 512], F32, tag="mm")
                for hc in range(HC):
                    nc.tensor.matmul(ps[:], wq_t[hc][:, INNER + ic * 128:INNER + (ic + 1) * 128],
                                     xnT[hc][:], start=(hc == 0), stop=(hc == 7))
                kt = lnsp.tile([128, SM], BF16, tag="ktl")
                nc.vector.tensor_scalar_add(kt[:], ps[:], qkvb_pp[:, 8 + ic:9 + ic])
                nc.sync.dma_start(out=ag_in_a[ic * 128:(ic + 1) * 128, :],
                                  in_=kt[:, 0:256])
                nc.sync.dma_start(out=ag_in_b[ic * 128:(ic + 1) * 128, :],
                                  in_=kt[:, 256:512])

            # ---- V^T, transpose to rows with interleaved ones columns ----
            v_loc = [lnp.tile([128, VW], BF16, tag=f"vloc{si}", name=f"vloc{si}")
                     for si in range(4)]
            for si in range(4):
                nc.gpsimd.memset(v_loc[si], 1.0)
            for ic in range(IC):
                ps = ps_qkv.tile([128, 512], F32, tag="mm")
                for hc in range(HC):
                    nc.tensor.matmul(ps[:], wq_t[hc][:, 2 * INNER + ic * 128:2 * INNER + (ic + 1) * 128],
                                     xnT[hc][:], start=(hc == 0), stop=(hc == 7))
                vtmp = lnsp.tile([128, SM], F32, tag="vtmp")
                nc.vector.tensor_scalar_add(vtmp[:], ps[:], qkvb_pp[:, 16 + ic:17 + ic])
                h0, h1 = 2 * ic, 2 * ic + 1
                for si in range(4):
                    pst = ps_qkv.tile([128, 512], F32, tag="tps", bufs=2)
                    nc.tensor.transpose(pst[:, 0:128],
                                        vtmp[:, si * 128:(si + 1) * 128], ident[:])
                    nc.vector.tensor_copy(v_loc[si][:, h0 * 66:h0 * 66 + 64],
                                          pst[:, 0:64])
                    nc.vector.tensor_copy(v_loc[si][:, h1 * 66:h1 * 66 + 64],
                                          pst[:, 64:128])
            for si in range(2):
                nc.sync.dma_start(
                    out=ag_in_a[INNER + si * 528:INNER + (si + 1) * 528, :],
                    in_=v_loc[si][:])
            nc.gpsimd.collective_compute(
                "AllGather", ALU.bypass, replica_groups=[list(range(NC))],
                ins=[ag_in_a.opt()], outs=[ag_out_a.opt()])
            for si in range(2):
                nc.sync.dma_start(
                    out=ag_in_b[INNER + si * 528:INNER + (si + 1) * 528, :],
                    in_=v_loc[2 + si][:])
            nc.gpsimd.collective_compute(
                "AllGather", ALU.bypass, replica_groups=[list(range(NC))],
                ins=[ag_in_b.opt()], outs=[ag_out_b.opt()])

            # ---- Q^T while the AllGather is in flight ----
            for ic in range(IC):
                ps = ps_qkv.tile([128, 512], F32, tag="mm")
                for hc in range(HC):
                    nc.tensor.matmul(ps[:], wq_t[hc][:, ic * 128:(ic + 1) * 128],
                                     xnT[hc][:], start=(hc == 0), stop=(hc == 7))
                for h in (2 * ic, 2 * ic + 1):
                    hs = slice((h % 2) * 64, (h % 2) * 64 + 64)
                    nc.vector.tensor_scalar_add(QT[h][hs, :], ps[hs, :],
                                                qkvb_pp[hs, ic:ic + 1])
        setp.release()

        # ---- attention (my 512 queries, all heads, all keys) ----
        with tc.tile_pool(name="kvg", bufs=1) as kvg, \
             tc.tile_pool(name="expp", bufs=4) as expp, \
             tc.tile_pool(name="rcp", bufs=2) as rcp, \
             tc.tile_pool(name="ps_mm", bufs=3, space="PSUM") as ps_mm, \
             tc.tile_pool(name="ps_acc", bufs=1, space="PSUM") as ps_acc, \
             tc.tile_pool(name="ps_bc", bufs=1, space="PSUM") as ps_bc:

            KT_g, V_g = {}, {}
            for half, ago in ((0, ag_out_a), (1, ag_out_b)):
                for c in range(NC):
                    for ic in range(IC):
                        t = kvg.tile([128, 256], BF16, tag=f"ktg{half}_{c}_{ic}",
                                     name=f"ktg{half}_{c}_{ic}")
                        nc.sync.dma_start(out=t[:],
                                          in_=ago[c, ic * 128:(ic + 1) * 128, :])
                        KT_g[(half, c, ic)] = t
                    for sj in range(2):
                        si = half * 2 + sj
                        t = kvg.tile([128, VW], BF16, tag=f"vg{c}_{si}",
                                     name=f"vg{c}_{si}")
                        nc.sync.dma_start(
                            out=t[:],
                            in_=ago[c, INNER + sj * 528:INNER + (sj + 1) * 528, :])
                        V_g[(c, si)] = t

            heads = list(range(NH))
            exi = 0
            for g in range(0, NH, 4):
                grp = heads[g:g + 4]
                cps_l = [ps_acc.tile([65, 512], F32, name=f"cps{g}_{i}", tag=f"cps{i}")
                         for i in range(len(grp))]
                for kt in range(32):
                    half, b2 = kt // 16, kt % 2
                    c, blk = (kt % 16) // 2, (kt // 16) * 2 + kt % 2
                    exl = []
                    for h, cps in zip(grp, cps_l):
                        sps = ps_mm.tile([128, 512], F32, tag="mm")
                        nc.tensor.matmul(sps[:],
                                         KT_g[(half, c, h // 2)][:, b2 * 128:(b2 + 1) * 128],
                                         QT[h][:], start=True, stop=True)
                        if exi % 2 == 0:
                            ex = expp.tile([128, 512], BF16, tag="exp")
                            nc.scalar.activation(ex[:], sps[:], AF.Exp, scale=0.125)
                            exl.append(ex[:])
                        else:
                            exq = expp.tile([128, 512], I16, tag="expi")
                            nc.vector.tensor_scalar(
                                out=exq[:], in0=sps[:], scalar1=EXP_MUL,
                                scalar2=EXP_BIAS, op0=ALU.mult, op1=ALU.add)
                            exl.append(exq[:].bitcast(BF16))
                        exi += 1
                    for h, cps, ex in zip(grp, cps_l, exl):
                        nc.tensor.matmul(cps[:], V_g[(c, blk)][:, h * 66:h * 66 + 65],
                                         ex, start=(kt == 0), stop=(kt == 31),
                                         skip_group_check=True)
                for h, cps in zip(grp, cps_l):
                    ic, off = h // 2, (h % 2) * 64
                    dn = rcp.tile([1, 512], F32, tag="dn")
                    nc.vector.tensor_copy(dn[:], cps[64:65, :])
                    rc = rcp.tile([1, 512], F32, tag="rc")
                    nc.vector.reciprocal_approx_fast(rc[:], dn[:])
                    bps = ps_bc.tile([128, 512], F32, tag="rbc")
                    nc.tensor.matmul(bps[0:64, :], ones_f[0:1, 0:64], rc[0:1, :],
                                     start=True, stop=True)
                    bsb = rcp.tile([64, 512], F32, tag="bsb", bufs=1)
                    nc.vector.tensor_copy(bsb[:], bps[0:64, :])
                    nc.vector.tensor_mul(ctxT[ic][off:off + 64, :], cps[0:64, :], bsb[:])
        qkvp.release()

        # ---- out-proj + residual + LN2 (row layout, no collective) ----
        owp = tc.alloc_tile_pool(name="owp", bufs=1)
        owT_sb = []
        for ic in range(IC):
            t = owp.tile([128, H], BF16, tag=f"owT{ic}")
            nc.sync.dma_start(out=t[:], in_=owT[ic * 128:(ic + 1) * 128, :])
            owT_sb.append(t)
        ffp = tc.alloc_tile_pool(name="ffp", bufs=1)
        with tc.tile_pool(name="ffs", bufs=3) as ffsp, \
             tc.tile_pool(name="w1p", bufs=1) as w1p, \
             tc.tile_pool(name="w2p", bufs=8) as w2p, \
             tc.tile_pool(name="ps_f1", bufs=2, space="PSUM") as ps_f1, \
             tc.tile_pool(name="ps_tr", bufs=2, space="PSUM") as ps_tr, \
             tc.tile_pool(name="ps_f2", bufs=1, space="PSUM") as ps_f2:

            # prefetch all FF1 weights during out-proj/LN2
            w1t = {}
            for ib in range(8):
                for hc in range(HC):
                    t = w1p.tile([128, 512], BF16, tag=f"w1_{ib}_{hc}")
                    nc.sync.dma_start(
                        out=t[:],
                        in_=ff1wT[hc * 128:(hc + 1) * 128, ib * 512:(ib + 1) * 512])
                    w1t[(ib, hc)] = t

            ln2p = tc.alloc_tile_pool(name="ln2p", bufs=2)
            xn2T = [ffp.tile([128, SM], BF16, tag=f"xn2T{hc}", name=f"xn2T{hc}") for hc in range(HC)]
            x2_tiles = []
            for st in range(4):
                x2 = ffp.tile([128, H], F32, tag=f"x2{st}")
                for hf in range(2):
                    yo = ps_f1.tile([128, 512], F32, tag="f1")
                    for ic in range(IC):
                        nc.tensor.matmul(yo[:], ctxT[ic][:, st * 128:(st + 1) * 128],
                                         owT_sb[ic][:, hf * 512:(hf + 1) * 512],
                                         start=(ic == 0), stop=(ic == 7))
                    sl = slice(hf * 512, (hf + 1) * 512)
                    nc.vector.tensor_add(x2[:, sl], yo[:], xm_tiles[st][:, sl])
                    nc.vector.tensor_add(x2[:, sl], x2[:, sl], ob_bc[:, sl])
                x2_tiles.append(x2)
                ns = ffsp.tile([128, 1], F32, tag="negsum2")
                nc.vector.reduce_sum(out=ns[:], in_=x2[:], axis=AXX, negate=True)
                nm = ffsp.tile([128, 1], F32, tag="negmean2")
                nc.scalar.mul(nm[:], ns[:], 1.0 / H)
                xn2 = ln2p.tile([128, H], F32, tag="xn2", name="xn2")
                nc.vector.scalar_tensor_tensor(
                    out=xn2[:], in0=x2[:], scalar=nm[:], in1=fnw_bc[:],
                    op0=ALU.add, op1=ALU.mult)
                nc.vector.tensor_add(xn2[:], xn2[:], fnb_bc[:])
                for hc in range(HC):
                    ps = ps_tr.tile([128, 512], F32, tag="tr")
                    nc.tensor.transpose(ps[:, 0:128],
                                        xn2[:, hc * 128:(hc + 1) * 128],
                                        ident[:])
                    nc.vector.tensor_copy(xn2T[hc][:, st * 128:(st + 1) * 128],
                                          ps[:, 0:128])
            ln2p.release()

            hT = [ffp.tile([128, SM], BF16, tag=f"hT{i}", name=f"hT{i}") for i in range(32)]
            for ib in range(8):
                for sub in range(4):
                    it = ib * 4 + sub
                    ps = ps_f1.tile([128, 512], F32, tag="f1")
                    for hc in range(HC):
                        nc.tensor.matmul(ps[:],
                                         w1t[(ib, hc)][:, sub * 128:(sub + 1) * 128],
                                         xn2T[hc][:], start=(hc == 0), stop=(hc == 7))
                    nc.scalar.activation(hT[it][:], ps[:], AF.Relu,
                                         bias=ffb1_pp[:, it:it + 1])

            for hf in range(2):
                yps = [ps_f2.tile([128, 512], F32, name=f"yps{hf}_{i}", tag=f"yps{i}", bufs=1)
                       for i in range(4)]
                for ic in range(32):
                    w2t = w2p.tile([128, 512], BF16, tag="w2")
                    nc.sync.dma_start(
                        out=w2t[:],
                        in_=ff2wT[ic * 128:(ic + 1) * 128, hf * 512:(hf + 1) * 512])
                    for s4 in range(4):
                        nc.tensor.matmul(yps[s4][:],
                                         hT[ic][:, s4 * 128:(s4 + 1) * 128],
                                         w2t[:], start=(ic == 0), stop=(ic == 31),
                                         skip_group_check=True)
                for s4 in range(4):
                    sl = slice(hf * 512, (hf + 1) * 512)
                    ysb = ffsp.tile([128, 512], F32, tag="ysb", name="ysb")
                    nc.vector.tensor_add(ysb[:], yps[s4][:], x2_tiles[s4][:, sl])
                    nc.vector.tensor_add(ysb[:], ysb[:], ffb2_bc[:, sl])
                    nc.sync.dma_start(out=y[s4 * 128:(s4 + 1) * 128, sl], in_=ysb[:])

        ffp.release()
        owp.release()
        ctxp.release()
        xmp.release()
        dram.release()
        cst.release()

    nc.compile()
    return nc


def make_in_maps(inputs):
    f = lambda a: np.ascontiguousarray(np.asarray(a, dtype=np.float32))
    bf = mybir.dt.np(BF16)
    x = f(inputs["x"])
    q_w, k_w, v_w = f(inputs["q_w"]), f(inputs["k_w"]), f(inputs["v_w"])
    o_w = f(inputs["o_w"])
    ff1_w, ff2_w = f(inputs["ff1_w"]), f(inputs["ff2_w"])
    wqkvT = np.ascontiguousarray(
        np.concatenate([q_w.T, k_w.T, v_w.T], axis=1).astype(bf))
    bqkv = np.ascontiguousarray(np.concatenate(
        [f(inputs["q_b"]), f(inputs["k_b"]), f(inputs["v_b"])]).reshape(24, 128))
    owT = np.ascontiguousarray(o_w.T.astype(bf))
    ff1wT = np.ascontiguousarray(ff1_w.T.astype(bf))
    ff2wT = np.ascontiguousarray(ff2_w.T.astype(bf))
    ff1b = np.ascontiguousarray(f(inputs["ff1_b"]).reshape(32, 128))
    row = lambda a: np.ascontiguousarray(a.reshape(1, -1))
    shared = {
        "wqkvT": wqkvT, "bqkv": bqkv, "owT": owT,
        "ob": row(f(inputs["o_b"])),
        "anw": row(f(inputs["an_w"])), "anb": row(f(inputs["an_b"])),
        "fnw": row(f(inputs["fn_w"])), "fnb": row(f(inputs["fn_b"])),
        "ff1wT": ff1wT, "ff1b": ff1b,
        "ff2wT": ff2wT, "ffb2": row(f(inputs["ff2_b"])),
    }
    in_maps = []
    for m in range(NC):
        d = dict(shared)
        d["x_m"] = np.ascontiguousarray(x[m * SM:(m + 1) * SM])
        in_maps.append(d)
    return in_maps


def kernel(**inputs) -> np.ndarray:
    from concourse.bass_utils import run_bass_kernel_spmd
    if "nc" not in _CACHE:
        _CACHE["nc"] = build_nc()
    nc = _CACHE["nc"]
    in_maps = make_in_maps(inputs)
    res = run_bass_kernel_spmd(nc, in_maps, core_ids=list(range(NC)))
    return np.concatenate([res.results[m]["y"] for m in range(NC)], axis=0)


# revision 21
# speedup vs baseline: 1.0301x; 1.0301x over previous
"""BertBlock (mean-only LN, 16-head attention, relu FF) on 8 trn2 NeuronCores.

Sharding: sequence-parallel. Each core owns S/8 = 512 rows end-to-end:
LN1, QKV projections (ALL 16 heads for its rows), attention for its 512
queries against all 4096 keys, out-proj, residual, LN2 and the FF — all
local. The only collective is ONE bf16 AllGather carrying K^T and V
(V stored row-major with an interleaved ones-column per head so the
softmax denominator falls out of the same PE accumulation). Q^T is
computed while the AllGather is in flight. No ReduceScatter.

exp() is split between the Scalar engine (exact) and the Vector engine
(Schraudolph fast-exp via int16 bit arithmetic, ~3% rel err) so neither
engine gates the PE. Matmuls run bf16 (1 cycle/row); residuals f32.
"""
import sys

sys.path.insert(0, '/opt/trn_rl_repo')

import numpy as np
import concourse.bass as bass
from concourse import bacc
import concourse.mybir as mybir
import concourse.tile as tile
from concourse.masks import make_identity

S = 4096          # sequence length
H = 1024          # hidden
I_ = 4096         # ffn inner
NH = 16           # heads
HD = 64           # head dim
INNER = NH * HD   # 1024
NC = 8            # cores
SM = S // NC      # 512 rows per core
HC = H // 128     # 8 hidden chunks
IC = INNER // 128 # 8 inner chunks
VW = NH * (HD + 2)  # 1056: per head: 64 V cols, ones col, pad col
AGR = INNER + SM * VW // 512  # 2080 rows of the merged AllGather buffer
F32 = mybir.dt.float32
F32R = mybir.dt.float32r
BF16 = mybir.dt.bfloat16
I16 = mybir.dt.int16
AF = mybir.ActivationFunctionType
ALU = mybir.AluOpType
AXX = mybir.AxisListType.X

# Schraudolph fast-exp in bf16 bit space: bf16_bits(exp(s*x)) ~=
# floor(x * (s*128/ln2) + B); B tuned for minimax rel err (~3.3%),
# including the DVE's truncating f32->int16 conversion.
EXP_MUL = 23.083120654223414   # 0.125 * 128 / ln(2)
EXP_BIAS = 16250.89

_CACHE = {}


def build_nc():
    nc = bacc.Bacc(None, target_bir_lowering=False, debug=False)
    P = lambda name, shape, dt=F32: nc.declare_dram_parameter(name, shape, dt, isOutput=False)
    x_m = P("x_m", [SM, H])
    wqkvT = P("wqkvT", [H, 3 * INNER], BF16)   # [h, q|k|v inner cols, all heads]
    bqkv = P("bqkv", [24, 128])                # (3*INNER) reshaped
    owT = P("owT", [INNER, H], BF16)           # o_w.T
    ob = P("ob", [1, H])
    anw, anb = P("anw", [1, H]), P("anb", [1, H])
    fnw, fnb = P("fnw", [1, H]), P("fnb", [1, H])
    ff1wT = P("ff1wT", [H, I_], BF16)
    ff1b = P("ff1b", [32, 128])
    ff2wT = P("ff2wT", [I_, H], BF16)
    ffb2 = P("ffb2", [1, H])
    y = nc.declare_dram_parameter("y", [SM, H], F32, isOutput=True)

    with tile.TileContext(nc) as tc:
        cst = tc.alloc_tile_pool(name="cst", bufs=1)
        dram = tc.alloc_tile_pool(name="dram", bufs=1, space="DRAM")
        xmp = tc.alloc_tile_pool(name="xmp", bufs=1)
        ctxp = tc.alloc_tile_pool(name="ctxp", bufs=1)
        ctxT = [ctxp.tile([128, SM], BF16, tag=f"ctxT{i}", name=f"ctxT{i}")
                for i in range(IC)]
        qkvp = tc.alloc_tile_pool(name="qkvp", bufs=1)
        QT = [qkvp.tile([128, SM], BF16, tag=f"QT{i}", name=f"QT{i}") for i in range(NH)]
        setp = tc.alloc_tile_pool(name="setp", bufs=1)
        ps_set = tc.alloc_tile_pool(name="ps_set", bufs=2, space="PSUM")

        # merged AllGather buffer: rows 0:1024 = K^T, rows 1024:2080 = V flat
        ag_in = dram.tile([AGR, 512], BF16)
        ag_out = dram.tile([NC, AGR, 512], BF16, addr_space="Shared")

        # ---- constants ----
        ident = cst.tile([128, 128], F32)
        make_identity(nc, ident)
        ones_f = cst.tile([1, 128], F32)
        nc.gpsimd.memset(ones_f, 1.0)
        ones1 = cst.tile([1, 128], F32R)
        nc.vector.tensor_copy(ones1[:], ones_f[:])
        for h in range(NH):
            z = slice(64, 128) if h % 2 == 0 else slice(0, 64)
            nc.gpsimd.memset(QT[h][z, :], 0.0)

        def load_vec(p):
            t = setp.tile([1, H], F32, tag=f"v_{p.name}")
            nc.sync.dma_start(out=t[:], in_=p[:])
            return t

        vecs = {n: load_vec(p) for n, p in
                [("anw", anw), ("anb", anb), ("fnw", fnw), ("fnb", fnb),
                 ("ob", ob), ("ffb2", ffb2)]}

        def bcast(name, pool=None):
            # [1, H] -> [128, H] broadcast across partitions via PE
            v = vecs[name]
            bc = (pool or cst).tile([128, H], F32, tag=f"bc_{name}", name=f"bc_{name}")
            for hf in range(H // 512):
                ps = ps_set.tile([128, 512], F32)
                nc.tensor.matmul(ps[:], ones_f[0:1, :], v[0:1, hf * 512:(hf + 1) * 512],
                                 start=True, stop=True)
                nc.vector.tensor_copy(bc[:, hf * 512:(hf + 1) * 512], ps[:])
            return bc

        anw_bc, anb_bc = bcast("anw", setp), bcast("anb", setp)
        fnw_bc, fnb_bc = bcast("fnw"), bcast("fnb")
        ob_bc, ffb2_bc = bcast("ob"), bcast("ffb2")

        # qkv bias, per-partition layout: [128, 24] (col j = inner block j)
        bqkv_ld = setp.tile([24, 128], F32)
        nc.sync.dma_start(out=bqkv_ld[:], in_=bqkv[:])
        ps = ps_set.tile([128, 512], F32)
        nc.tensor.transpose(ps[:, 0:24], bqkv_ld[:], ident[0:24, 0:24])
        qkvb_pp = cst.tile([128, 24], F32)
        nc.vector.tensor_copy(qkvb_pp[:], ps[:, 0:24])

        ffb1_ld = setp.tile([32, 128], F32)
        nc.sync.dma_start(out=ffb1_ld[:], in_=ff1b[:])
        ps = ps_set.tile([128, 512], F32)
        nc.tensor.transpose(ps[:, 0:32], ffb1_ld[:], ident[0:32, 0:32])
        ffb1_pp = cst.tile([128, 32], F32)
        nc.vector.tensor_copy(ffb1_pp[:], ps[:, 0:32])
        ps_set.release()

        # ---- load x, LN1 ----
        xm_tiles = []
        for i in range(4):
            t = xmp.tile([128, H], F32, tag=f"xm{i}")
            nc.sync.dma_start(out=t[:], in_=x_m[i * 128:(i + 1) * 128, :])
            xm_tiles.append(t)

        with tc.tile_pool(name="ln1", bufs=1) as lnp, \
             tc.tile_pool(name="ln1s", bufs=3) as lnsp, \
             tc.tile_pool(name="wq", bufs=1) as wqp, \
             tc.tile_pool(name="ps_qkv", bufs=3, space="PSUM") as ps_qkv:
            wq_t = []
            for hc in range(HC):
                t = wqp.tile([128, 3 * INNER], BF16, tag=f"wqkv{hc}")
                nc.sync.dma_start(out=t[:], in_=wqkvT[hc * 128:(hc + 1) * 128, :])
                wq_t.append(t)

            xn_tiles = []
            for i in range(4):
                ns = lnsp.tile([128, 1], F32, tag="negsum")
                nc.vector.reduce_sum(out=ns[:], in_=xm_tiles[i][:], axis=AXX,
                                     negate=True)
                nm = lnsp.tile([128, 1], F32, tag="negmean")
                nc.scalar.mul(nm[:], ns[:], 1.0 / H)
                xn = lnp.tile([128, H], F32, tag=f"xn{i}")
                nc.vector.scalar_tensor_tensor(
                    out=xn[:], in0=xm_tiles[i][:], scalar=nm[:], in1=anw_bc[:],
                    op0=ALU.add, op1=ALU.mult)
                nc.vector.tensor_add(xn[:], xn[:], anb_bc[:])
                xn_tiles.append(xn)
            # transpose -> xnT [h, s] bf16 (si-outer: start as soon as each
            # LN1 row-tile is done)
            xnT = [lnp.tile([128, SM], BF16, tag=f"xnT{hc}", name=f"xnT{hc}")
                   for hc in range(HC)]
            for si in range(4):
                for hc in range(HC):
                    ps = ps_qkv.tile([128, 512], F32, tag="tps", bufs=2)
                    nc.tensor.transpose(ps[:, 0:128],
                                        xn_tiles[si][:, hc * 128:(hc + 1) * 128],
                                        ident[:])
                    nc.vector.tensor_copy(xnT[hc][:, si * 128:(si + 1) * 128],
                                          ps[:, 0:128])

            # ---- K^T first (feeds the AllGather ASAP) ----
            for ic in range(IC):
                ps = ps_qkv.tile([128, 512], F32, tag="mm")
                for hc in range(HC):
                    nc.tensor.matmul(ps[:], wq_t[hc][:, INNER + ic * 128:INNER + (ic + 1) * 128],
                                     xnT[hc][:], start=(hc == 0), stop=(hc == 7))
                kt = lnsp.tile([128, SM], BF16, tag="ktl")
                nc.vector.tensor_scalar_add(kt[:], ps[:], qkvb_pp[:, 8 + ic:9 + ic])
                nc.sync.dma_start(out=ag_in[ic * 128:(ic + 1) * 128, :], in_=kt[:])

            # ---- V^T, transpose to rows with interleaved ones columns ----
            v_loc = [lnp.tile([128, VW], BF16, tag=f"vloc{si}", name=f"vloc{si}")
                     for si in range(4)]
            for si in range(4):
                nc.gpsimd.memset(v_loc[si], 1.0)
            for ic in range(IC):
                ps = ps_qkv.tile([128, 512], F32, tag="mm")
                for hc in range(HC):
                    nc.tensor.matmul(ps[:], wq_t[hc][:, 2 * INNER + ic * 128:2 * INNER + (ic + 1) * 128],
                                     xnT[hc][:], start=(hc == 0), stop=(hc == 7))
                vtmp = lnsp.tile([128, SM], F32, tag="vtmp")
                nc.vector.tensor_scalar_add(vtmp[:], ps[:], qkvb_pp[:, 16 + ic:17 + ic])
                h0, h1 = 2 * ic, 2 * ic + 1
                for si in range(4):
                    pst = ps_qkv.tile([128, 512], F32, tag="tps", bufs=2)
                    nc.tensor.transpose(pst[:, 0:128],
                                        vtmp[:, si * 128:(si + 1) * 128], ident[:])
                    nc.vector.tensor_copy(v_loc[si][:, h0 * 66:h0 * 66 + 64],
                                          pst[:, 0:64])
                    nc.vector.tensor_copy(v_loc[si][:, h1 * 66:h1 * 66 + 64],
                                          pst[:, 64:128])
            for si in range(4):
                nc.sync.dma_start(
                    out=ag_in[INNER + si * 264:INNER + (si + 1) * 264, :],
                    in_=v_loc[si][:])
            nc.gpsimd.collective_compute(
                "AllGather", ALU.bypass, replica_groups=[list(range(NC))],
                ins=[ag_in.opt()], outs=[ag_out.opt()])

            # ---- Q^T while the AllGather is in flight ----
            for ic in range(IC):
                ps = ps_qkv.tile([128, 512], F32, tag="mm")
                for hc in range(HC):
                    nc.tensor.matmul(ps[:], wq_t[hc][:, ic * 128:(ic + 1) * 128],
                                     xnT[hc][:], start=(hc == 0), stop=(hc == 7))
                for h in (2 * ic, 2 * ic + 1):
                    hs = slice((h % 2) * 64, (h % 2) * 64 + 64)
                    nc.vector.tensor_scalar_add(QT[h][hs, :], ps[hs, :],
                                                qkvb_pp[hs, ic:ic + 1])
        setp.release()

        # ---- attention (my 512 queries, all heads, all keys) ----
        with tc.tile_pool(name="kvg", bufs=1) as kvg, \
             tc.tile_pool(name="expp", bufs=4) as expp, \
             tc.tile_pool(name="rcp", bufs=2) as rcp, \
             tc.tile_pool(name="ps_mm", bufs=3, space="PSUM") as ps_mm, \
             tc.tile_pool(name="ps_acc", bufs=1, space="PSUM") as ps_acc, \
             tc.tile_pool(name="ps_bc", bufs=1, space="PSUM") as ps_bc:

            KT_g, V_g = {}, {}
            for c in range(NC):
                for ic in range(IC):
                    t = kvg.tile([128, SM], BF16, tag=f"ktg{c}_{ic}", name=f"ktg{c}_{ic}")
                    nc.sync.dma_start(out=t[:], in_=ag_out[c, ic * 128:(ic + 1) * 128, :])
                    KT_g[(c, ic)] = t
                for si in range(4):
                    t = kvg.tile([128, VW], BF16, tag=f"vg{c}_{si}", name=f"vg{c}_{si}")
                    nc.sync.dma_start(
                        out=t[:],
                        in_=ag_out[c, INNER + si * 264:INNER + (si + 1) * 264, :])
                    V_g[(c, si)] = t

            heads = list(range(NH))
            exi = 0
            for g in range(0, NH, 4):
                grp = heads[g:g + 4]
                cps_l = [ps_acc.tile([65, 512], F32, name=f"cps{g}_{i}", tag=f"cps{i}")
                         for i in range(len(grp))]
                for kt in range(32):
                    c, blk = kt // 4, kt % 4
                    exl = []
                    for h, cps in zip(grp, cps_l):
                        sps = ps_mm.tile([128, 512], F32, tag="mm")
                        nc.tensor.matmul(sps[:],
                                         KT_g[(c, h // 2)][:, blk * 128:(blk + 1) * 128],
                                         QT[h][:], start=True, stop=True)
                        if exi % 2 == 0:
                            ex = expp.tile([128, 512], BF16, tag="exp")
                            nc.scalar.activation(ex[:], sps[:], AF.Exp, scale=0.125)
                            exl.append(ex[:])
                        else:
                            exq = expp.tile([128, 512], I16, tag="expi")
                            nc.vector.tensor_scalar(
                                out=exq[:], in0=sps[:], scalar1=EXP_MUL,
                                scalar2=EXP_BIAS, op0=ALU.mult, op1=ALU.add)
                            exl.append(exq[:].bitcast(BF16))
                        exi += 1
                    for h, cps, ex in zip(grp, cps_l, exl):
                        nc.tensor.matmul(cps[:], V_g[(c, blk)][:, h * 66:h * 66 + 65],
                                         ex, start=(kt == 0), stop=(kt == 31),
                                         skip_group_check=True)
                for h, cps in zip(grp, cps_l):
                    ic, off = h // 2, (h % 2) * 64
                    dn = rcp.tile([1, 512], F32, tag="dn")
                    nc.vector.tensor_copy(dn[:], cps[64:65, :])
                    rc = rcp.tile([1, 512], F32, tag="rc")
                    nc.vector.reciprocal_approx_fast(rc[:], dn[:])
                    bps = ps_bc.tile([128, 512], F32, tag="rbc")
                    nc.tensor.matmul(bps[0:64, :], ones_f[0:1, 0:64], rc[0:1, :],
                                     start=True, stop=True)
                    bsb = rcp.tile([64, 512], F32, tag="bsb", bufs=1)
                    nc.vector.tensor_copy(bsb[:], bps[0:64, :])
                    nc.vector.tensor_mul(ctxT[ic][off:off + 64, :], cps[0:64, :], bsb[:])
        qkvp.release()

        # ---- out-proj + residual + LN2 (row layout, no collective) ----
        owp = tc.alloc_tile_pool(name="owp", bufs=1)
        owT_sb = []
        for ic in range(IC):
            t = owp.tile([128, H], BF16, tag=f"owT{ic}")
            nc.sync.dma_start(out=t[:], in_=owT[ic * 128:(ic + 1) * 128, :])
            owT_sb.append(t)
        ffp = tc.alloc_tile_pool(name="ffp", bufs=1)
        with tc.tile_pool(name="ffs", bufs=3) as ffsp, \
             tc.tile_pool(name="w1p", bufs=1) as w1p, \
             tc.tile_pool(name="w2p", bufs=8) as w2p, \
             tc.tile_pool(name="ps_f1", bufs=2, space="PSUM") as ps_f1, \
             tc.tile_pool(name="ps_tr", bufs=2, space="PSUM") as ps_tr, \
             tc.tile_pool(name="ps_f2", bufs=1, space="PSUM") as ps_f2:

            # prefetch all FF1 weights during out-proj/LN2
            w1t = {}
            for ib in range(8):
                for hc in range(HC):
                    t = w1p.tile([128, 512], BF16, tag=f"w1_{ib}_{hc}")
                    nc.sync.dma_start(
                        out=t[:],
                        in_=ff1wT[hc * 128:(hc + 1) * 128, ib * 512:(ib + 1) * 512])
                    w1t[(ib, hc)] = t

            ln2p = tc.alloc_tile_pool(name="ln2p", bufs=2)
            xn2T = [ffp.tile([128, SM], BF16, tag=f"xn2T{hc}", name=f"xn2T{hc}") for hc in range(HC)]
            x2_tiles = []
            for st in range(4):
                x2 = ffp.tile([128, H], F32, tag=f"x2{st}")
                for hf in range(2):
                    yo = ps_f1.tile([128, 512], F32, tag="f1")
                    for ic in range(IC):
                        nc.tensor.matmul(yo[:], ctxT[ic][:, st * 128:(st + 1) * 128],
                                         owT_sb[ic][:, hf * 512:(hf + 1) * 512],
                                         start=(ic == 0), stop=(ic == 7))
                    sl = slice(hf * 512, (hf + 1) * 512)
                    nc.vector.tensor_add(x2[:, sl], yo[:], xm_tiles[st][:, sl])
                    nc.vector.tensor_add(x2[:, sl], x2[:, sl], ob_bc[:, sl])
                x2_tiles.append(x2)
                ns = ffsp.tile([128, 1], F32, tag="negsum2")
                nc.vector.reduce_sum(out=ns[:], in_=x2[:], axis=AXX, negate=True)
                nm = ffsp.tile([128, 1], F32, tag="negmean2")
                nc.scalar.mul(nm[:], ns[:], 1.0 / H)
                xn2 = ln2p.tile([128, H], F32, tag="xn2", name="xn2")
                nc.vector.scalar_tensor_tensor(
                    out=xn2[:], in0=x2[:], scalar=nm[:], in1=fnw_bc[:],
                    op0=ALU.add, op1=ALU.mult)
                nc.vector.tensor_add(xn2[:], xn2[:], fnb_bc[:])
                for hc in range(HC):
                    ps = ps_tr.tile([128, 512], F32, tag="tr")
                    nc.tensor.transpose(ps[:, 0:128],
                                        xn2[:, hc * 128:(hc + 1) * 128],
                                        ident[:])
                    nc.vector.tensor_copy(xn2T[hc][:, st * 128:(st + 1) * 128],
                                          ps[:, 0:128])
            ln2p.release()

            hT = [ffp.tile([128, SM], BF16, tag=f"hT{i}", name=f"hT{i}") for i in range(32)]
            for ib in range(8):
                for sub in range(4):
                    it = ib * 4 + sub
                    ps = ps_f1.tile([128, 512], F32, tag="f1")
                    for hc in range(HC):
                        nc.tensor.matmul(ps[:],
                                         w1t[(ib, hc)][:, sub * 128:(sub + 1) * 128],
                                         xn2T[hc][:], start=(hc == 0), stop=(hc == 7))
                    nc.scalar.activation(hT[it][:], ps[:], AF.Relu,
                                         bias=ffb1_pp[:, it:it + 1])

            for hf in range(2):
                yps = [ps_f2.tile([128, 512], F32, name=f"yps{hf}_{i}", tag=f"yps{i}", bufs=1)
                       for i in range(4)]
                for ic in range(32):
                    w2t = w2p.tile([128, 512], BF16, tag="w2")
                    nc.sync.dma_start(
                        out=w2t[:],
                        in_=ff2wT[ic * 128:(ic + 1) * 128, hf * 512:(hf + 1) * 512])
                    for s4 in range(4):
                        nc.tensor.matmul(yps[s4][:],
                                         hT[ic][:, s4 * 128:(s4 + 1) * 128],
                                         w2t[:], start=(ic == 0), stop=(ic == 31),
                                         skip_group_check=True)
                for s4 in range(4):
                    sl = slice(hf * 512, (hf + 1) * 512)
                    ysb = ffsp.tile([128, 512], F32, tag="ysb", name="ysb")
                    nc.vector.tensor_add(ysb[:], yps[s4][:], x2_tiles[s4][:, sl])
                    nc.vector.tensor_add(ysb[:], ysb[:], ffb2_bc[:, sl])
                    nc.sync.dma_start(out=y[s4 * 128:(s4 + 1) * 128, sl], in_=ysb[:])

        ffp.release()
        owp.release()
        ctxp.release()
        xmp.release()
        dram.release()
        cst.release()

    nc.compile()
    return nc


def make_in_maps(inputs):
    f = lambda a: np.ascontiguousarray(np.asarray(a, dtype=np.float32))
    bf = mybir.dt.np(BF16)
    x = f(inputs["x"])
    q_w, k_w, v_w = f(inputs["q_w"]), f(inputs["k_w"]), f(inputs["v_w"])
    o_w = f(inputs["o_w"])
    ff1_w, ff2_w = f(inputs["ff1_w"]), f(inputs["ff2_w"])
    wqkvT = np.ascontiguousarray(
        np.concatenate([q_w.T, k_w.T, v_w.T], axis=1).astype(bf))
    bqkv = np.ascontiguousarray(np.concatenate(
        [f(inputs["q_b"]), f(inputs["k_b"]), f(inputs["v_b"])]).reshape(24, 128))
    owT = np.ascontiguousarray(o_w.T.astype(bf))
    ff1wT = np.ascontiguousarray(ff1_w.T.astype(bf))
    ff2wT = np.ascontiguousarray(ff2_w.T.astype(bf))
    ff1b = np.ascontiguousarray(f(inputs["ff1_b"]).reshape(32, 128))
    row = lambda a: np.ascontiguousarray(a.reshape(1, -1))
    shared = {
        "wqkvT": wqkvT, "bqkv": bqkv, "owT": owT,
        "ob": row(f(inputs["o_b"])),
        "anw": row(f(inputs["an_w"])), "anb": row(f(inputs["an_b"])),
        "fnw": row(f(inputs["fn_w"])), "fnb": row(f(inputs["fn_b"])),
        "ff1wT": ff1wT, "ff1b": ff1b,
        "ff2wT": ff2wT, "ffb2": row(f(inputs["ff2_b"])),
    }
    in_maps = []
    for m in range(NC):
        d = dict(shared)
        d["x_m"] = np.ascontiguousarray(x[m * SM:(m + 1) * SM])
        in_maps.append(d)
    return in_maps


def kernel(**inputs) -> np.ndarray:
    from concourse.bass_utils import run_bass_kernel_spmd
    if "nc" not in _CACHE:
        _CACHE["nc"] = build_nc()
    nc = _CACHE["nc"]
    in_maps = make_in_maps(inputs)
    res = run_bass_kernel_spmd(nc, in_maps, core_ids=list(range(NC)))
    return np.concatenate([res.results[m]["y"] for m in range(NC)], axis=0)


# revision 22
# speedup vs baseline: 1.0447x; 1.0142x over previous
"""BertBlock (mean-only LN, 16-head attention, relu FF) on 8 trn2 NeuronCores.

Sharding: sequence-parallel. Each core owns S/8 = 512 rows end-to-end:
LN1, QKV projections (ALL 16 heads for its rows), attention for its 512
queries against all 4096 keys, out-proj, residual, LN2 and the FF — all
local. The only collective is ONE bf16 AllGather carrying K^T and V
(V stored row-major with an interleaved ones-column per head so the
softmax denominator falls out of the same PE accumulation). Q^T is
computed while the AllGather is in flight. No ReduceScatter.

exp() is split between the Scalar engine (exact) and the Vector engine
(Schraudolph fast-exp via int16 bit arithmetic, ~3% rel err) so neither
engine gates the PE. Matmuls run bf16 (1 cycle/row); residuals f32.
"""
import sys

sys.path.insert(0, '/opt/trn_rl_repo')

import numpy as np
import concourse.bass as bass
from concourse import bacc
import concourse.mybir as mybir
import concourse.tile as tile
from concourse.masks import make_identity

S = 4096          # sequence length
H = 1024          # hidden
I_ = 4096         # ffn inner
NH = 16           # heads
HD = 64           # head dim
INNER = NH * HD   # 1024
NC = 8            # cores
SM = S // NC      # 512 rows per core
HC = H // 128     # 8 hidden chunks
IC = INNER // 128 # 8 inner chunks
VW = NH * (HD + 2)  # 1056: per head: 64 V cols, ones col, pad col
AGR = INNER + SM * VW // 512  # 2080 rows of the merged AllGather buffer
F32 = mybir.dt.float32
F32R = mybir.dt.float32r
BF16 = mybir.dt.bfloat16
I16 = mybir.dt.int16
AF = mybir.ActivationFunctionType
ALU = mybir.AluOpType
AXX = mybir.AxisListType.X

# Schraudolph fast-exp in bf16 bit space: bf16_bits(exp(s*x)) ~=
# floor(x * (s*128/ln2) + B); B tuned for minimax rel err (~3.3%),
# including the DVE's truncating f32->int16 conversion.
EXP_MUL = 23.083120654223414   # 0.125 * 128 / ln(2)
EXP_BIAS = 16250.89

_CACHE = {}


def build_nc():
    nc = bacc.Bacc(None, target_bir_lowering=False, debug=False)
    P = lambda name, shape, dt=F32: nc.declare_dram_parameter(name, shape, dt, isOutput=False)
    x_m = P("x_m", [SM, H])
    wqkvT = P("wqkvT", [H, 3 * INNER], BF16)   # [h, q|k|v inner cols, all heads]
    bqkv = P("bqkv", [24, 128])                # (3*INNER) reshaped
    owT = P("owT", [INNER, H], BF16)           # o_w.T
    ob = P("ob", [1, H])
    anw, anb = P("anw", [1, H]), P("anb", [1, H])
    fnw, fnb = P("fnw", [1, H]), P("fnb", [1, H])
    ff1wT = P("ff1wT", [H, I_], BF16)
    ff1b = P("ff1b", [32, 128])
    ff2wT = P("ff2wT", [I_, H], BF16)
    ffb2 = P("ffb2", [1, H])
    y = nc.declare_dram_parameter("y", [SM, H], F32, isOutput=True)

    with tile.TileContext(nc) as tc:
        cst = tc.alloc_tile_pool(name="cst", bufs=1)
        dram = tc.alloc_tile_pool(name="dram", bufs=1, space="DRAM")
        xmp = tc.alloc_tile_pool(name="xmp", bufs=1)
        ctxp = tc.alloc_tile_pool(name="ctxp", bufs=1)
        ctxT = [ctxp.tile([128, SM], BF16, tag=f"ctxT{i}", name=f"ctxT{i}")
                for i in range(IC)]
        qkvp = tc.alloc_tile_pool(name="qkvp", bufs=1)
        QT = [qkvp.tile([128, SM], BF16, tag=f"QT{i}", name=f"QT{i}") for i in range(NH)]
        setp = tc.alloc_tile_pool(name="setp", bufs=1)
        ps_set = tc.alloc_tile_pool(name="ps_set", bufs=2, space="PSUM")

        # merged AllGather buffer: rows 0:1024 = K^T, rows 1024:2080 = V flat
        ag_in = dram.tile([AGR, 512], BF16)
        ag_out = dram.tile([NC, AGR, 512], BF16, addr_space="Shared")

        # ---- constants ----
        ident = cst.tile([128, 128], F32)
        make_identity(nc, ident)
        ones_f = cst.tile([1, 128], F32)
        nc.gpsimd.memset(ones_f, 1.0)
        ones1 = cst.tile([1, 128], F32R)
        nc.vector.tensor_copy(ones1[:], ones_f[:])
        for h in range(NH):
            z = slice(64, 128) if h % 2 == 0 else slice(0, 64)
            nc.gpsimd.memset(QT[h][z, :], 0.0)

        def load_vec(p):
            t = setp.tile([1, H], F32, tag=f"v_{p.name}")
            nc.sync.dma_start(out=t[:], in_=p[:])
            return t

        vecs = {n: load_vec(p) for n, p in
                [("anw", anw), ("anb", anb), ("fnw", fnw), ("fnb", fnb),
                 ("ob", ob), ("ffb2", ffb2)]}

        def bcast(name, pool=None):
            # [1, H] -> [128, H] broadcast across partitions via PE
            v = vecs[name]
            bc = (pool or cst).tile([128, H], F32, tag=f"bc_{name}", name=f"bc_{name}")
            for hf in range(H // 512):
                ps = ps_set.tile([128, 512], F32)
                nc.tensor.matmul(ps[:], ones_f[0:1, :], v[0:1, hf * 512:(hf + 1) * 512],
                                 start=True, stop=True)
                nc.vector.tensor_copy(bc[:, hf * 512:(hf + 1) * 512], ps[:])
            return bc

        anw_bc, anb_bc = bcast("anw", setp), bcast("anb", setp)
        fnw_bc, fnb_bc = bcast("fnw"), bcast("fnb")
        ob_bc, ffb2_bc = bcast("ob"), bcast("ffb2")

        # qkv bias, per-partition layout: [128, 24] (col j = inner block j)
        bqkv_ld = setp.tile([24, 128], F32)
        nc.sync.dma_start(out=bqkv_ld[:], in_=bqkv[:])
        ps = ps_set.tile([128, 512], F32)
        nc.tensor.transpose(ps[:, 0:24], bqkv_ld[:], ident[0:24, 0:24])
        qkvb_pp = cst.tile([128, 24], F32)
        nc.vector.tensor_copy(qkvb_pp[:], ps[:, 0:24])

        ffb1_ld = setp.tile([32, 128], F32)
        nc.sync.dma_start(out=ffb1_ld[:], in_=ff1b[:])
        ps = ps_set.tile([128, 512], F32)
        nc.tensor.transpose(ps[:, 0:32], ffb1_ld[:], ident[0:32, 0:32])
        ffb1_pp = cst.tile([128, 32], F32)
        nc.vector.tensor_copy(ffb1_pp[:], ps[:, 0:32])
        ps_set.release()

        # ---- load x, LN1 ----
        xm_tiles = []
        for i in range(4):
            t = xmp.tile([128, H], F32, tag=f"xm{i}")
            nc.sync.dma_start(out=t[:], in_=x_m[i * 128:(i + 1) * 128, :])
            xm_tiles.append(t)

        with tc.tile_pool(name="ln1", bufs=1) as lnp, \
             tc.tile_pool(name="ln1s", bufs=3) as lnsp, \
             tc.tile_pool(name="wq", bufs=1) as wqp, \
             tc.tile_pool(name="ps_qkv", bufs=3, space="PSUM") as ps_qkv:
            wq_t = []
            for hc in range(HC):
                t = wqp.tile([128, 3 * INNER], BF16, tag=f"wqkv{hc}")
                nc.sync.dma_start(out=t[:], in_=wqkvT[hc * 128:(hc + 1) * 128, :])
                wq_t.append(t)

            xn_tiles = []
            for i in range(4):
                ns = lnsp.tile([128, 1], F32, tag="negsum")
                nc.vector.reduce_sum(out=ns[:], in_=xm_tiles[i][:], axis=AXX,
                                     negate=True)
                nm = lnsp.tile([128, 1], F32, tag="negmean")
                nc.scalar.mul(nm[:], ns[:], 1.0 / H)
                xn = lnp.tile([128, H], F32, tag=f"xn{i}")
                nc.vector.scalar_tensor_tensor(
                    out=xn[:], in0=xm_tiles[i][:], scalar=nm[:], in1=anw_bc[:],
                    op0=ALU.add, op1=ALU.mult)
                nc.vector.tensor_add(xn[:], xn[:], anb_bc[:])
                xn_tiles.append(xn)
            # transpose -> xnT [h, s] bf16
            xnT = []
            for hc in range(HC):
                xt = lnp.tile([128, SM], BF16, tag=f"xnT{hc}")
                for si in range(4):
                    ps = ps_qkv.tile([128, 512], F32, tag="tps", bufs=2)
                    nc.tensor.transpose(ps[:, 0:128],
                                        xn_tiles[si][:, hc * 128:(hc + 1) * 128],
                                        ident[:])
                    nc.vector.tensor_copy(xt[:, si * 128:(si + 1) * 128],
                                          ps[:, 0:128])
                xnT.append(xt)

            # ---- K^T first (feeds the AllGather ASAP) ----
            for ic in range(IC):
                ps = ps_qkv.tile([128, 512], F32, tag="mm")
                for hc in range(HC):
                    nc.tensor.matmul(ps[:], wq_t[hc][:, INNER + ic * 128:INNER + (ic + 1) * 128],
                                     xnT[hc][:], start=(hc == 0), stop=(hc == 7))
                kt = lnsp.tile([128, SM], BF16, tag="ktl")
                nc.vector.tensor_scalar_add(kt[:], ps[:], qkvb_pp[:, 8 + ic:9 + ic])
                nc.sync.dma_start(out=ag_in[ic * 128:(ic + 1) * 128, :], in_=kt[:])

            # ---- V^T, transpose to rows with interleaved ones columns ----
            v_loc = [lnp.tile([128, VW], BF16, tag=f"vloc{si}", name=f"vloc{si}")
                     for si in range(4)]
            for si in range(4):
                nc.gpsimd.memset(v_loc[si], 1.0)
            for ic in range(IC):
                ps = ps_qkv.tile([128, 512], F32, tag="mm")
                for hc in range(HC):
                    nc.tensor.matmul(ps[:], wq_t[hc][:, 2 * INNER + ic * 128:2 * INNER + (ic + 1) * 128],
                                     xnT[hc][:], start=(hc == 0), stop=(hc == 7))
                vtmp = lnsp.tile([128, SM], F32, tag="vtmp")
                nc.vector.tensor_scalar_add(vtmp[:], ps[:], qkvb_pp[:, 16 + ic:17 + ic])
                h0, h1 = 2 * ic, 2 * ic + 1
                for si in range(4):
                    pst = ps_qkv.tile([128, 512], F32, tag="tps", bufs=2)
                    nc.tensor.transpose(pst[:, 0:128],
                                        vtmp[:, si * 128:(si + 1) * 128], ident[:])
                    nc.vector.tensor_copy(v_loc[si][:, h0 * 66:h0 * 66 + 64],
                                          pst[:, 0:64])
                    nc.vector.tensor_copy(v_loc[si][:, h1 * 66:h1 * 66 + 64],
                                          pst[:, 64:128])
            for si in range(4):
                nc.sync.dma_start(
                    out=ag_in[INNER + si * 264:INNER + (si + 1) * 264, :],
                    in_=v_loc[si][:])
            nc.gpsimd.collective_compute(
                "AllGather", ALU.bypass, replica_groups=[list(range(NC))],
                ins=[ag_in.opt()], outs=[ag_out.opt()])

            # ---- Q^T while the AllGather is in flight ----
            for ic in range(IC):
                ps = ps_qkv.tile([128, 512], F32, tag="mm")
                for hc in range(HC):
                    nc.tensor.matmul(ps[:], wq_t[hc][:, ic * 128:(ic + 1) * 128],
                                     xnT[hc][:], start=(hc == 0), stop=(hc == 7))
                for h in (2 * ic, 2 * ic + 1):
                    hs = slice((h % 2) * 64, (h % 2) * 64 + 64)
                    nc.vector.tensor_scalar_add(QT[h][hs, :], ps[hs, :],
                                                qkvb_pp[hs, ic:ic + 1])
        setp.release()

        # ---- attention (my 512 queries, all heads, all keys) ----
        with tc.tile_pool(name="kvg", bufs=1) as kvg, \
             tc.tile_pool(name="expp", bufs=4) as expp, \
             tc.tile_pool(name="rcp", bufs=2) as rcp, \
             tc.tile_pool(name="ps_mm", bufs=3, space="PSUM") as ps_mm, \
             tc.tile_pool(name="ps_acc", bufs=1, space="PSUM") as ps_acc, \
             tc.tile_pool(name="ps_bc", bufs=1, space="PSUM") as ps_bc:

            KT_g, V_g = {}, {}
            for c in range(NC):
                for ic in range(IC):
                    t = kvg.tile([128, SM], BF16, tag=f"ktg{c}_{ic}", name=f"ktg{c}_{ic}")
                    nc.sync.dma_start(out=t[:], in_=ag_out[c, ic * 128:(ic + 1) * 128, :])
                    KT_g[(c, ic)] = t
                for si in range(4):
                    t = kvg.tile([128, VW], BF16, tag=f"vg{c}_{si}", name=f"vg{c}_{si}")
                    nc.sync.dma_start(
                        out=t[:],
                        in_=ag_out[c, INNER + si * 264:INNER + (si + 1) * 264, :])
                    V_g[(c, si)] = t

            heads = list(range(NH))
            exi = 0
            for g in range(0, NH, 4):
                grp = heads[g:g + 4]
                cps_l = [ps_acc.tile([65, 512], F32, name=f"cps{g}_{i}", tag=f"cps{i}")
                         for i in range(len(grp))]
                for kt in range(32):
                    c, blk = kt // 4, kt % 4
                    exl = []
                    for h, cps in zip(grp, cps_l):
                        sps = ps_mm.tile([128, 512], F32, tag="mm")
                        nc.tensor.matmul(sps[:],
                                         KT_g[(c, h // 2)][:, blk * 128:(blk + 1) * 128],
                                         QT[h][:], start=True, stop=True)
                        if exi % 2 == 0:
                            ex = expp.tile([128, 512], BF16, tag="exp")
                            nc.scalar.activation(ex[:], sps[:], AF.Exp, scale=0.125)
                            exl.append(ex[:])
                        else:
                            exq = expp.tile([128, 512], I16, tag="expi")
                            nc.vector.tensor_scalar(
                                out=exq[:], in0=sps[:], scalar1=EXP_MUL,
                                scalar2=EXP_BIAS, op0=ALU.mult, op1=ALU.add)
                            exl.append(exq[:].bitcast(BF16))
                        exi += 1
                    for h, cps, ex in zip(grp, cps_l, exl):
                        nc.tensor.matmul(cps[:], V_g[(c, blk)][:, h * 66:h * 66 + 65],
                                         ex, start=(kt == 0), stop=(kt == 31),
                                         skip_group_check=True)
                for h, cps in zip(grp, cps_l):
                    ic, off = h // 2, (h % 2) * 64
                    dn = rcp.tile([1, 512], F32, tag="dn")
                    nc.vector.tensor_copy(dn[:], cps[64:65, :])
                    rc = rcp.tile([1, 512], F32, tag="rc")
                    nc.vector.reciprocal_approx_fast(rc[:], dn[:])
                    bps = ps_bc.tile([128, 512], F32, tag="rbc")
                    nc.tensor.matmul(bps[0:64, :], ones_f[0:1, 0:64], rc[0:1, :],
                                     start=True, stop=True)
                    bsb = rcp.tile([64, 512], F32, tag="bsb", bufs=1)
                    nc.vector.tensor_copy(bsb[:], bps[0:64, :])
                    nc.vector.tensor_mul(ctxT[ic][off:off + 64, :], cps[0:64, :], bsb[:])
        qkvp.release()

        # ---- out-proj + residual + LN2 (row layout, no collective) ----
        owp = tc.alloc_tile_pool(name="owp", bufs=1)
        owT_sb = []
        for ic in range(IC):
            t = owp.tile([128, H], BF16, tag=f"owT{ic}")
            nc.sync.dma_start(out=t[:], in_=owT[ic * 128:(ic + 1) * 128, :])
            owT_sb.append(t)
        ffp = tc.alloc_tile_pool(name="ffp", bufs=1)
        with tc.tile_pool(name="ffs", bufs=3) as ffsp, \
             tc.tile_pool(name="w1p", bufs=1) as w1p, \
             tc.tile_pool(name="w2p", bufs=8) as w2p, \
             tc.tile_pool(name="ps_f1", bufs=2, space="PSUM") as ps_f1, \
             tc.tile_pool(name="ps_tr", bufs=2, space="PSUM") as ps_tr, \
             tc.tile_pool(name="ps_f2", bufs=1, space="PSUM") as ps_f2:

            # prefetch all FF1 weights during out-proj/LN2
            w1t = {}
            for ib in range(8):
                for hc in range(HC):
                    t = w1p.tile([128, 512], BF16, tag=f"w1_{ib}_{hc}")
                    nc.sync.dma_start(
                        out=t[:],
                        in_=ff1wT[hc * 128:(hc + 1) * 128, ib * 512:(ib + 1) * 512])
                    w1t[(ib, hc)] = t

            ln2p = tc.alloc_tile_pool(name="ln2p", bufs=2)
            xn2T = [ffp.tile([128, SM], BF16, tag=f"xn2T{hc}", name=f"xn2T{hc}") for hc in range(HC)]
            x2_tiles = []
            for st in range(4):
                x2 = ffp.tile([128, H], F32, tag=f"x2{st}")
                for hf in range(2):
                    yo = ps_f1.tile([128, 512], F32, tag="f1")
                    for ic in range(IC):
                        nc.tensor.matmul(yo[:], ctxT[ic][:, st * 128:(st + 1) * 128],
                                         owT_sb[ic][:, hf * 512:(hf + 1) * 512],
                                         start=(ic == 0), stop=(ic == 7))
                    sl = slice(hf * 512, (hf + 1) * 512)
                    nc.vector.tensor_add(x2[:, sl], yo[:], xm_tiles[st][:, sl])
                    nc.vector.tensor_add(x2[:, sl], x2[:, sl], ob_bc[:, sl])
                x2_tiles.append(x2)
                ns = ffsp.tile([128, 1], F32, tag="negsum2")
                nc.vector.reduce_sum(out=ns[:], in_=x2[:], axis=AXX, negate=True)
                nm = ffsp.tile([128, 1], F32, tag="negmean2")
                nc.scalar.mul(nm[:], ns[:], 1.0 / H)
                xn2 = ln2p.tile([128, H], F32, tag="xn2", name="xn2")
                nc.vector.scalar_tensor_tensor(
                    out=xn2[:], in0=x2[:], scalar=nm[:], in1=fnw_bc[:],
                    op0=ALU.add, op1=ALU.mult)
                nc.vector.tensor_add(xn2[:], xn2[:], fnb_bc[:])
                for hc in range(HC):
                    ps = ps_tr.tile([128, 512], F32, tag="tr")
                    nc.tensor.transpose(ps[:, 0:128],
                                        xn2[:, hc * 128:(hc + 1) * 128],
                                        ident[:])
                    nc.vector.tensor_copy(xn2T[hc][:, st * 128:(st + 1) * 128],
                                          ps[:, 0:128])
            ln2p.release()

            hT = [ffp.tile([128, SM], BF16, tag=f"hT{i}", name=f"hT{i}") for i in range(32)]
            for ib in range(8):
                for sub in range(4):
                    it = ib * 4 + sub
                    ps = ps_f1.tile([128, 512], F32, tag="f1")
                    for hc in range(HC):
                        nc.tensor.matmul(ps[:],
                                         w1t[(ib, hc)][:, sub * 128:(sub + 1) * 128],
                                         xn2T[hc][:], start=(hc == 0), stop=(hc == 7))
                    nc.scalar.activation(hT[it][:], ps[:], AF.Relu,
                                         bias=ffb1_pp[:, it:it + 1])

            for hf in range(2):
                yps = [ps_f2.tile([128, 512], F32, name=f"yps{hf}_{i}", tag=f"yps{i}", bufs=1)
                       for i in range(4)]
                for ic in range(32):
                    w2t = w2p.tile([128, 512], BF16, tag="w2")
                    nc.sync.dma_start(
                        out=w2t[:],
                        in_=ff2wT[ic * 128:(ic + 1) * 128, hf * 512:(hf + 1) * 512])
                    for s4 in range(4):
                        nc.tensor.matmul(yps[s4][:],
                                         hT[ic][:, s4 * 128:(s4 + 1) * 128],
                                         w2t[:], start=(ic == 0), stop=(ic == 31),
                                         skip_group_check=True)
                for s4 in range(4):
                    sl = slice(hf * 512, (hf + 1) * 512)
                    ysb = ffsp.tile([128, 512], F32, tag="ysb", name="ysb")
                    nc.vector.tensor_add(ysb[:], yps[s4][:], x2_tiles[s4][:, sl])
                    nc.vector.tensor_add(ysb[:], ysb[:], ffb2_bc[:, sl])
                    nc.sync.dma_start(out=y[s4 * 128:(s4 + 1) * 128, sl], in_=ysb[:])

        ffp.release()
        owp.release()
        ctxp.release()
        xmp.release()
        dram.release()
        cst.release()

    nc.compile()
    return nc


def make_in_maps(inputs):
    f = lambda a: np.ascontiguousarray(np.asarray(a, dtype=np.float32))
    bf = mybir.dt.np(BF16)
    x = f(inputs["x"])
    q_w, k_w, v_w = f(inputs["q_w"]), f(inputs["k_w"]), f(inputs["v_w"])
    o_w = f(inputs["o_w"])
    ff1_w, ff2_w = f(inputs["ff1_w"]), f(inputs["ff2_w"])
    wqkvT = np.ascontiguousarray(
        np.concatenate([q_w.T, k_w.T, v_w.T], axis=1).astype(bf))
    bqkv = np.ascontiguousarray(np.concatenate(
        [f(inputs["q_b"]), f(inputs["k_b"]), f(inputs["v_b"])]).reshape(24, 128))
    owT = np.ascontiguousarray(o_w.T.astype(bf))
    ff1wT = np.ascontiguousarray(ff1_w.T.astype(bf))
    ff2wT = np.ascontiguousarray(ff2_w.T.astype(bf))
    ff1b = np.ascontiguousarray(f(inputs["ff1_b"]).reshape(32, 128))
    row = lambda a: np.ascontiguousarray(a.reshape(1, -1))
    shared = {
        "wqkvT": wqkvT, "bqkv": bqkv, "owT": owT,
        "ob": row(f(inputs["o_b"])),
        "anw": row(f(inputs["an_w"])), "anb": row(f(inputs["an_b"])),
        "fnw": row(f(inputs["fn_w"])), "fnb": row(f(inputs["fn_b"])),
        "ff1wT": ff1wT, "ff1b": ff1b,
        "ff2wT": ff2wT, "ffb2": row(f(inputs["ff2_b"])),
    }
    in_maps = []
    for m in range(NC):
        d = dict(shared)
        d["x_m"] = np.ascontiguousarray(x[m * SM:(m + 1) * SM])
        in_maps.append(d)
    return in_maps


def kernel(**inputs) -> np.ndarray:
    from concourse.bass_utils import run_bass_kernel_spmd
    if "nc" not in _CACHE:
        _CACHE["nc"] = build_nc()
    nc = _CACHE["nc"]
    in_maps = make_in_maps(inputs)
    res = run_bass_kernel_spmd(nc, in_maps, core_ids=list(range(NC)))
    return np.concatenate([res.results[m]["y"] for m in range(NC)], axis=0)
